# revision 12
# baseline (speedup 1.0000x reference)
"""Tropical min-max matmul kernel for Trainium2.

out[b, o] = min_i max(x[b, i], weight[i, o])   with  x: [1024, 512], weight: [512, 512], fp32.

Strategy (v5)
-------------
Data-parallel over the batch dim: 8 NeuronCores x 128 rows of x each; weight
replicated (no collectives). Tolerance is 2e-2 rel err and min/max only
*select* values, so the whole kernel runs in fp16 (error = fp16 input
rounding, ~5e-4).

Per core, weight is held transposed (wT[o, i], o on partitions in 4 row
blocks) so the contraction axis i is the DVE free axis. Batch rows are
processed in groups of G=16 through a 4-engine pipeline:

  SYNC  stages each group's 16 x rows contiguously onto SBUF partition 0
        (one small chunked DMA per group, double-buffered).
  PE    broadcasts each row across all 128 partitions with a matmul whose
        stationary is a fixed ones[1, 128] (out[o, f] = rhs[0, f]); the
        moving operand walks the staged rows via a register-offset AP.
        Each 4-matmul quad ends with drain().then_inc so the PSUM writes
        are retired before the consumer reads.
  ACT   drains PSUM -> SBUF fp16 four banks at a time.
  DVE   per group: one wide tensor_tensor(max) over [128, G*4*512] (fp16
        packed => 2x mode, measured 0.53 ns/elem on HW), then a pairwise
        tensor_tensor(min) halving tree (512->16, in place, 2x mode) and
        a final tensor_reduce(min, X) over 8 into the output tile.

Everything per-group runs inside hardware Fori loops with register-carried
AP offsets and register-valued semaphore waits (first two groups are a
static prologue). Measured on this HW, each *distinct* instruction word in
a long unrolled stream costs ~10us (PE: ~57us) to issue, which dominated
earlier versions; loop bodies replay from the sequencer cheaply. A
stationary matmul re-load costs ~57us, so the ones matrix never changes; a
stride-0 "broadcast DMA" from HBM runs at ~9 GB/s on one queue and is also
far too slow.

Measured DVE group time ~42us -> ~340us/core/pass expected; PE (~19us) and
ACT (~14us) per group hide under it. Result tile ot[128, 4*128] =
[o-within-block, block*128+b] is reassembled on the host.
"""

import os
import sys

for _p in ("/opt/trn_rl_repo", "/root/.axon_site/_ro/trn_rl_repo"):
    if os.path.isdir(_p) and _p not in sys.path:
        sys.path.insert(0, _p)

import numpy as np

import concourse.bass as bass
import concourse.mybir as mybir
from concourse.bass_utils import run_bass_kernel_spmd

B, I, O = 1024, 512, 512
NCORES = 8
BS = B // NCORES   # 128 batch rows per core
OBLK = O // 128    # 4 output-feature blocks

TRACE = False
LAST_RESULTS = None
BENCH = 0
BENCH_TIMES = None

_F32 = mybir.dt.float32
_F16 = mybir.dt.float16

# "fp16" (fast, ~5e-4 rel err) or "fp32" (exact, debug)
DTYPE_MODE = os.environ.get("MINMAX_DTYPE", "fp16")


def _build_nc_wide(dt, detect_races=True, repeat=1, group=16):
    """v5: Fori-looped fixed-stationary PE broadcast + fp16 DVE max/min-tree."""
    nc = bass.Bass(detect_race_conditions=detect_races)
    G = group
    NGRP = BS // G
    W = G * OBLK * I           # scratch free size per group
    NBANK = 512                # fp32 elems per PSUM bank
    SLOT = G * I               # elems per xf/bc slot
    PE = mybir.EngineType.PE
    DVE = mybir.EngineType.DVE
    ACT = mybir.EngineType.Activation
    SP = mybir.EngineType.SP

    xd = nc.declare_dram_parameter("x", [BS, I], dt, isOutput=False)
    wt_d = nc.declare_dram_parameter("wT", [O, I], dt, isOutput=False)
    ones_d = nc.declare_dram_parameter("ones", [1, 128], dt, isOutput=False)
    out_d = nc.declare_dram_parameter("ot", [128, OBLK * BS], dt, isOutput=True)

    with (
        nc.sbuf_tensor([128, OBLK * I], dt) as wt_sb,
        # xf/bc/ot are padded past their live region: register-offset APs are
        # bounds-checked against conservative register ranges ([0, mod-1] from
        # reg_mod plus walker advances), which overshoot the live extent.
        nc.sbuf_tensor([1, 3 * SLOT], dt) as xf_sb,
        nc.sbuf_tensor([1, 128], dt) as ones_sb,
        nc.sbuf_tensor([128, 3 * SLOT], dt) as bc_sb,
        nc.sbuf_tensor([128, W], dt) as scr_sb,
        nc.sbuf_tensor([128, OBLK * BS + 128], dt) as ot_sb,
        nc.psum_tensor([128, 8 * NBANK], _F32) as ps,
        nc.semaphore("dma_sem") as dma_sem,
        nc.semaphore("pe_sem") as pe_sem,    # counts drained PE quads
        nc.semaphore("act_sem") as act_sem,  # counts completed ACT drains
        nc.semaphore("v_sem") as v_sem,      # counts completed DVE groups
    ):
        NB = repeat * NGRP
        assert NGRP >= 2

        wt_v = wt_sb[:, :]
        scr_v = scr_sb[:, :]
        xf_v = xf_sb[:, :]
        bc_v = bc_sb[:, :]
        ot_v = ot_sb[:, :]
        p_wt = wt_v.ap[0][0]
        p_scr = scr_v.ap[0][0]
        p_bc = bc_v.ap[0][0]
        p_ot = ot_v.ap[0][0]
        p_ps = ps[:, :].ap[0][0]

        # ---------------- SYNC: input DMAs + xf chunk staging ----------------
        nc.sync.dma_start(
            out=wt_sb[:, :].rearrange("p (t i) -> p t i", t=OBLK),
            in_=wt_d.rearrange("(t p) i -> p t i", p=128),
        ).then_inc(dma_sem, 16)
        nc.sync.dma_start(out=ones_sb[:, :], in_=ones_d[:, :]).then_inc(dma_sem, 16)

        def xf_chunk(src_off, dst_off):
            src = bass.AP(tensor=xd[:, :].tensor, offset=src_off,
                          ap=[[0, 1], [1, SLOT]])
            dst = bass.AP(tensor=xf_v.tensor, offset=dst_off,
                          ap=[[xf_v.ap[0][0], 1], [1, SLOT]])
            return nc.sync.dma_start(out=dst, in_=src).then_inc(dma_sem, 16)

        # prologue: repeat 0 (groups 0..NGRP-1), static.  Each chunk issue
        # waits for the previous DMA's completion: keeps dma_sem updates
        # totally ordered (race-detector requirement; ~8us/group, hidden).
        for gg in range(NGRP):
            if gg >= 1:
                nc.sync.wait_ge(dma_sem, 16 * (gg + 2))
            if gg >= 2:
                nc.sync.wait_ge(pe_sem, 4 * (gg - 1))
            xf_chunk((gg % NGRP) * SLOT, (gg % 2) * SLOT)
        # steady state: one body per repeat, NGRP static chunk DMAs
        if repeat > 1:
            rs_gate = nc.sync.alloc_register("rs_gate")   # 4*(gg-1)
            rs_dma = nc.sync.alloc_register("rs_dma")     # 16*(gg+2)
            nc.sync.reg_mov(rs_gate, 4 * (NGRP - 1))
            nc.sync.reg_mov(rs_dma, 16 * (NGRP + 2))
            with nc.Fori(1, repeat, engines={SP}):
                for c in range(NGRP):
                    nc.sync.wait_ge(dma_sem, rs_dma)
                    nc.sync.wait_ge(pe_sem, rs_gate)
                    xf_chunk(c * SLOT, (c % 2) * SLOT)
                    nc.sync.reg_add(rs_gate, rs_gate, 4)
                    nc.sync.reg_add(rs_dma, rs_dma, 16)
        nc.sync.wait_ge(v_sem, NB)
        nc.sync.wait_ge(dma_sem, 16 * (NB + 2))
        nc.sync.dma_start(out=out_d[:, :],
                          in_=ot_sb[:, 0:OBLK * BS]).then_inc(dma_sem, 16)
        nc.sync.wait_ge(dma_sem, 16 * (NB + 3))

        # ---------------- PE: ones-stationary row broadcast ----------------
        ones_v = ones_sb[:, :]

        def pe_quad(rhs_off_start, bank0, act_target, advance_reg=None):
            """4 matmuls into banks bank0..bank0+3, then drain+inc."""
            if act_target is not None:
                nc.tensor.wait_ge(act_sem, act_target)
            for k in range(4):
                if advance_reg is None:
                    rhs = bass.AP(tensor=xf_v.tensor,
                                  offset=rhs_off_start + k * I,
                                  ap=[[xf_v.ap[0][0], 1], [1, I]])
                else:
                    rhs = bass.AP(tensor=xf_v.tensor, offset=advance_reg,
                                  ap=[[xf_v.ap[0][0], 1], [1, I]])
                nc.tensor.matmul(
                    ps[:, (bank0 + k) * NBANK:(bank0 + k + 1) * NBANK],
                    ones_v, rhs, skip_group_check=True)
                if advance_reg is not None:
                    nc.tensor.reg_add(advance_reg, advance_reg, I)
            nc.tensor.drain().then_inc(pe_sem, 1)

        # prologue: repeat 0 (quads Q = 0..4*NGRP-1), static
        for gg in range(NGRP):
            nc.tensor.wait_ge(dma_sem, 16 * (gg + 3))
            for k in range(4):
                Q = 4 * gg + k
                pe_quad((gg % 2) * SLOT + k * 4 * I, (k % 2) * 4,
                        Q - 1 if Q >= 2 else None)
        # steady state: one body per HALF group (8 matmuls).  A single loop
        # body tolerates only ~12 register-offset matmuls before the symbolic
        # value tracker gives up, so per-group state advances via parity
        # toggle registers (t alternates 0 <-> step; x += t applies the step
        # every second body).
        if NB > NGRP:
            rp_dma = nc.tensor.alloc_register("rp_dma")    # 16*(gg+3)
            rp_t = nc.tensor.alloc_register("rp_t")        # 0 <-> 16
            rp_act = nc.tensor.alloc_register("rp_act")    # Q-1
            rp_slot = nc.tensor.alloc_register("rp_slot")  # (gg%2)*SLOT
            rp_t2 = nc.tensor.alloc_register("rp_t2")      # 0 <-> SLOT
            rp_half = nc.tensor.alloc_register("rp_half")  # 0 <-> SLOT//2
            rp_rhs = nc.tensor.alloc_register("rp_rhs")
            nc.tensor.reg_mov(rp_dma, 16 * (NGRP + 3))
            nc.tensor.reg_mov(rp_t, 0)
            nc.tensor.reg_mov(rp_act, 4 * NGRP - 1)
            nc.tensor.reg_mov(rp_slot, (NGRP % 2) * SLOT)
            nc.tensor.reg_mov(rp_t2, 0)
            nc.tensor.reg_mov(rp_half, 0)
            with nc.Fori(2 * NGRP, 2 * NB, engines={PE}):
                nc.tensor.wait_ge(dma_sem, rp_dma)
                nc.tensor.reg_mov(rp_rhs, rp_slot)
                nc.tensor.reg_add(rp_rhs, rp_rhs, rp_half)
                for q in range(2):
                    pe_quad(None, (q % 2) * 4, rp_act, advance_reg=rp_rhs)
                    nc.tensor.reg_add(rp_act, rp_act, 1)
                # dma target / slot advance every second body
                nc.tensor.reg_add(rp_dma, rp_dma, rp_t)
                nc.tensor.reg_mul(rp_t, rp_t, -1)
                nc.tensor.reg_add(rp_t, rp_t, 16)
                nc.tensor.reg_add(rp_slot, rp_slot, rp_t2)
                nc.tensor.reg_mod(rp_slot, rp_slot, 2 * SLOT)
                nc.tensor.reg_mul(rp_t2, rp_t2, -1)
                nc.tensor.reg_add(rp_t2, rp_t2, SLOT)
                nc.tensor.reg_mul(rp_half, rp_half, -1)
                nc.tensor.reg_add(rp_half, rp_half, SLOT // 2)

        # ---------------- ACT: PSUM -> bc drains ----------------
        def act_drain(bc_off, q, pe_target, reg_off=None):
            nc.scalar.wait_ge(pe_sem, pe_target)
            off = reg_off if reg_off is not None else bc_off
            dst = bass.AP(tensor=bc_v.tensor, offset=off,
                          ap=[[p_bc, 128], [1, 4 * I]])
            nc.scalar.copy(
                out=dst, in_=ps[:, q * 4 * NBANK:(q + 1) * 4 * NBANK]
            ).then_inc(act_sem, 1)

        for gg in range(NGRP):
            if gg >= 2:
                nc.scalar.wait_ge(v_sem, gg - 1)
            for k in range(4):
                d = 4 * gg + k
                act_drain((gg % 2) * SLOT + k * 4 * I, k % 2, d + 1)
        if NB > NGRP:
            ra_pe = nc.scalar.alloc_register("ra_pe")    # d+1
            ra_v = nc.scalar.alloc_register("ra_v")      # gg-1
            ra_slot = nc.scalar.alloc_register("ra_slot")
            ra_bc = nc.scalar.alloc_register("ra_bc")
            nc.scalar.reg_mov(ra_pe, 4 * NGRP + 1)
            nc.scalar.reg_mov(ra_v, NGRP - 1)
            nc.scalar.reg_mov(ra_slot, (NGRP % 2) * SLOT)
            with nc.Fori(NGRP, NB, engines={ACT}):
                nc.scalar.wait_ge(v_sem, ra_v)
                nc.scalar.reg_mov(ra_bc, ra_slot)
                for k in range(4):
                    act_drain(None, k % 2, ra_pe, reg_off=ra_bc)
                    nc.scalar.reg_add(ra_pe, ra_pe, 1)
                    nc.scalar.reg_add(ra_bc, ra_bc, 4 * I)
                nc.scalar.reg_add(ra_v, ra_v, 1)
                nc.scalar.reg_add(ra_slot, ra_slot, SLOT)
                nc.scalar.reg_mod(ra_slot, ra_slot, 2 * SLOT)

        # ---------------- DVE: max + min tree ----------------
        def dve_group(act_target, bc_off, ot_off):
            nc.vector.wait_ge(act_sem, act_target)
            in0 = bass.AP(tensor=wt_v.tensor, offset=0,
                          ap=[[p_wt, 128], [0, G], [I, OBLK], [1, I]])
            in1 = bass.AP(tensor=bc_v.tensor, offset=bc_off,
                          ap=[[p_bc, 128], [I, G], [0, OBLK], [1, I]])
            mx = bass.AP(tensor=scr_v.tensor, offset=0,
                         ap=[[p_scr, 128], [OBLK * I, G], [I, OBLK], [1, I]])
            nc.vector.tensor_tensor(out=mx, in0=in0, in1=in1,
                                    op=mybir.AluOpType.max)
            w = I
            while w > 8:
                h = w // 2
                lo = bass.AP(tensor=scr_v.tensor, offset=0,
                             ap=[[p_scr, 128], [OBLK * I, G], [I, OBLK], [1, h]])
                hi = bass.AP(tensor=scr_v.tensor, offset=h,
                             ap=[[p_scr, 128], [OBLK * I, G], [I, OBLK], [1, h]])
                nc.vector.tensor_tensor(out=lo, in0=lo, in1=hi,
                                        op=mybir.AluOpType.min)
                w = h
            red_in = bass.AP(tensor=scr_v.tensor, offset=0,
                             ap=[[p_scr, 128], [OBLK * I, G], [I, OBLK], [1, w]])
            red_out = bass.AP(tensor=ot_v.tensor, offset=ot_off,
                              ap=[[p_ot, 128], [1, G], [BS, OBLK]])
            nc.vector.tensor_reduce(out=red_out, in_=red_in,
                                    op=mybir.AluOpType.min,
                                    axis=mybir.AxisListType.X).then_inc(v_sem, 1)

        for gg in range(NGRP):
            dve_group(4 * (gg + 1), (gg % 2) * SLOT, (gg % NGRP) * G)
        if NB > NGRP:
            rv_act = nc.vector.alloc_register("rv_act")   # 4*(gg+1)
            rv_bc = nc.vector.alloc_register("rv_bc")     # (gg%2)*SLOT
            rv_ot = nc.vector.alloc_register("rv_ot")     # (gg%NGRP)*G
            nc.vector.reg_mov(rv_act, 4 * (NGRP + 1))
            nc.vector.reg_mov(rv_bc, (NGRP % 2) * SLOT)
            nc.vector.reg_mov(rv_ot, (NGRP % NGRP) * G)
            with nc.Fori(NGRP, NB, engines={DVE}):
                dve_group(rv_act, rv_bc, rv_ot)
                nc.vector.reg_add(rv_act, rv_act, 4)
                nc.vector.reg_add(rv_bc, rv_bc, SLOT)
                nc.vector.reg_mod(rv_bc, rv_bc, 2 * SLOT)
                nc.vector.reg_add(rv_ot, rv_ot, G)
                nc.vector.reg_mod(rv_ot, rv_ot, BS)

    return nc


_NC_CACHE = {}


def _get_nc(mode):
    if mode not in _NC_CACHE:
        if mode == "fp16":
            _NC_CACHE[mode] = _build_nc_wide(_F16, group=16)
        else:
            _NC_CACHE[mode] = _build_nc_wide(_F32, group=16)
    return _NC_CACHE[mode]


def _make_in_maps(x, weight, npdt):
    wt_h = np.ascontiguousarray(np.asarray(weight).T.astype(npdt))  # [O, I]
    xh = np.asarray(x).astype(npdt)
    ones = np.ones((1, 128), dtype=npdt)
    return [
        {
            "x": np.ascontiguousarray(xh[c * BS:(c + 1) * BS]),
            "wT": wt_h,
            "ones": ones,
        }
        for c in range(NCORES)
    ]


def kernel(x, weight):
    global LAST_RESULTS
    x = np.asarray(x)
    weight = np.asarray(weight)
    in_dtype = x.dtype

    mode = DTYPE_MODE
    npdt = np.float16 if mode == "fp16" else np.float32
    nc = _get_nc(mode)

    in_maps = _make_in_maps(x, weight, npdt)

    res = run_bass_kernel_spmd(nc, in_maps, list(range(NCORES)), trace=TRACE)
    LAST_RESULTS = res

    if BENCH > 0:
        import time as _time

        global BENCH_TIMES
        BENCH_TIMES = []
        for _ in range(BENCH):
            t0 = _time.perf_counter()
            run_bass_kernel_spmd(nc, in_maps, list(range(NCORES)), trace=False)
            BENCH_TIMES.append(_time.perf_counter() - t0)

    # ot[oo, t*BS + b] = out_core[b, t*128 + oo]
    parts = []
    for c in range(NCORES):
        ot = np.asarray(res.results[c]["ot"])          # [128, OBLK*BS]
        oc = ot.reshape(128, OBLK, BS).transpose(2, 1, 0).reshape(BS, O)
        parts.append(oc)
    out = np.concatenate(parts, axis=0)
    return out.astype(in_dtype)
